# revision 23
# baseline (speedup 1.0000x reference)
"""ConsistencyLoss Trainium2 kernel (8-core SPMD, anchor-sharded).

Shapes hardcoded for B=8, A=49104, C=20, M=32, N=16.

Sharding: each of the 8 cores owns a contiguous slice of 6138 anchors
(padded to 6144 = 16 blocks x 384) across ALL 8 samples.  This makes the
batch-summed alpha term local to each core; only 3 scalars per sample per
core come back to the host (npos, cls numerator, reg numerator).

Device layouts:
  IoU stage:  partition = (anchor_half h, box_slot s) = (2, 64),
              free = anchors (two half-passes of 1536).
              Box coords are per-partition scalars -> tensor_scalar 2x ops.
              Thresholds are division-free: iou>=t  <=>  inter >= t/(1+t)*(aA+aB).
  Slot contraction: PE matmuls sum indicator rows per sample -> hit counts
              and matched-box coords (exact: every anchor has <=1 hit >=0.5).
  CLS/REG stage: partition = (anchor_block, sample) = (16, 8), free = anchors.
"""

import math
from contextlib import ExitStack

import numpy as np

import concourse.bass as bass
import concourse.bacc as bacc
import concourse.tile as tile
from concourse import mybir
from concourse.bass_utils import run_bass_kernel_spmd

F32 = mybir.dt.float32
AX = mybir.AxisListType
OP = mybir.AluOpType
AF = mybir.ActivationFunctionType

B = 8
A = 49104
C = 20
NCORES = 8
ACORE = A // NCORES        # 6138 real anchors per core
ABLK = 16                  # anchor blocks per core
AIN = 384                  # anchors per block
APAD = ABLK * AIN          # 6144 padded anchors per core
NPADA = APAD - ACORE       # 6 pad anchors (tail of block 15)
NS = 64                    # box slots per slot-chunk
HB = ABLK // 2             # 8 blocks per partition-half
AHALF2 = 4 * AIN           # anchors per half-pass (4 blocks)
HB2 = 4                    # blocks per half-pass
EPS = 1e-4

_prog_cache = {}


def _bc(ap, pos, count):
    """Insert a stride-0 (broadcast) dim into an AP at position pos."""
    new = [list(d) for d in ap.ap]
    new.insert(pos, [0, count])
    return bass.AP(ap.tensor, ap.offset, new)


def _emit_iou(nc, iopool, wpool, ppool, pspool, anch16_t, selbc_t,
              slot_t, selred_t, red5, n_slot_chunks, hp):
    """One half-pass (blocks hp*4..hp*4+4 of each half): anchor broadcast,
    IoU indicators, PE slot contraction into red_t rows."""

    def io_tile(name, tag):
        return iopool.tile([128, AHALF2], F32, tag=tag, name=name)

    axc = [io_tile(f"axc{k}", f"axc{k}") for k in range(4)]
    thrhi = io_tile("thrhi", "thrhi")
    thrmid = io_tile("thrmid", "thrmid")
    areaB_sc = slot_t[:, 4:5]
    for ib in range(HB2):
        il = hp * HB2 + ib
        dst = slice(ib * AIN, (ib + 1) * AIN)
        for pl in range(5):
            ps = ppool.tile([128, AIN], F32, tag="psA")
            nc.tensor.matmul(
                ps[:],
                lhsT=selbc_t[:, il * 128:(il + 1) * 128],
                rhs=anch16_t[:, pl * AIN:(pl + 1) * AIN],
                start=True, stop=True,
            )
            if pl < 4:
                if pl % 2 == 0:
                    nc.scalar.copy(axc[pl][:, dst], ps[:])
                else:
                    nc.vector.tensor_copy(axc[pl][:, dst], ps[:])
            else:
                nc.vector.tensor_scalar(thrhi[:, dst], ps[:], areaB_sc,
                                        1.0 / 3.0, op0=OP.add, op1=OP.mult)
                nc.vector.tensor_scalar(thrmid[:, dst], ps[:], areaB_sc,
                                        2.0 / 7.0, op0=OP.add, op1=OP.mult)
    ax1R, ay1R, ax2R, ay2R = axc

    ind_tiles = []  # (ind_hi, ind_comb) per slot chunk
    for sc in range(n_slot_chunks):
        so = 5 * sc
        bx1 = slot_t[:, so + 0:so + 1]
        by1 = slot_t[:, so + 1:so + 2]
        bx2 = slot_t[:, so + 2:so + 3]
        by2 = slot_t[:, so + 3:so + 4]

        u1 = io_tile("u1", "t1")
        nc.vector.tensor_scalar(u1[:], ax2R[:], bx2, None, op0=OP.min)
        u2 = io_tile("u2", "t2")
        nc.vector.tensor_scalar(u2[:], ax1R[:], bx1, None, op0=OP.max)
        iwp = io_tile("iwp", "t3")
        nc.vector.scalar_tensor_tensor(iwp[:], u1[:], 1.0, u2[:],
                                       op0=OP.mult, op1=OP.subtract)
        v1 = io_tile("v1", "t4")
        nc.vector.tensor_scalar(v1[:], ay2R[:], by2, None, op0=OP.min)
        v2 = io_tile("v2", "t5")
        nc.vector.tensor_scalar(v2[:], ay1R[:], by1, None, op0=OP.max)
        ihp = io_tile("ihp", "t6")
        nc.vector.scalar_tensor_tensor(ihp[:], v1[:], 1.0, v2[:],
                                       op0=OP.mult, op1=OP.subtract)
        # ihr = relu(ihp) in place
        nc.scalar.activation(ihp[:], ihp[:], AF.Relu)
        # inter = relu(iwp) * ihr, in place on iwp
        nc.vector.scalar_tensor_tensor(iwp[:], iwp[:], 0.0, ihp[:],
                                       op0=OP.max, op1=OP.mult)
        inter = iwp
        ind_hi = u1  # reuse slot t1
        nc.vector.tensor_tensor(ind_hi[:], inter[:], thrhi[:], op=OP.is_ge)
        ind_mid = u2  # reuse slot t2
        nc.vector.tensor_tensor(ind_mid[:], inter[:], thrmid[:], op=OP.is_ge)
        # ind_comb = 64*ind_hi + ind_mid, in place on ind_mid
        nc.vector.scalar_tensor_tensor(ind_mid[:], ind_hi[:], 64.0,
                                       ind_mid[:], op0=OP.mult, op1=OP.add)
        ind_tiles.append((ind_hi, ind_mid))

    # groups: (plane index in red5/tmp5, which ind, selector plane)
    groups = [(0, 1, 0), (1, 0, 1), (2, 0, 2), (3, 0, 3), (4, 0, 4)]
    for h in range(2):
        # tmp5[j, plane, (i, a_in)] staging for all 5 reduction tensors
        tmp5 = wpool.tile([8, 5, HB2, AIN], F32, tag="redtmp", bufs=1)
        for gi, (plane, which, sel_plane) in enumerate(groups):
            # one 512-wide psum bank per 384-anchor block (bank aligned)
            ps = pspool.tile([8, HB2, 512], F32, tag="psR", bufs=1)
            for sc in range(n_slot_chunks):
                ind = ind_tiles[sc][which]
                sb = (sc * 5 + sel_plane) * 8
                sel = selred_t[64 * h:64 * h + 64, sb:sb + 8]
                for i in range(HB2):
                    nc.tensor.matmul(
                        ps[:, i, 0:AIN],
                        lhsT=sel,
                        rhs=ind[64 * h:64 * h + 64, i * AIN:(i + 1) * AIN],
                        start=(sc == 0),
                        stop=(sc == n_slot_chunks - 1),
                    )
            # PSUM -> SBUF staging (engines only)
            if gi % 2 == 0:
                nc.scalar.copy(tmp5[:, plane, :, :], ps[:, :, 0:AIN])
            else:
                nc.vector.tensor_copy(tmp5[:, plane, :, :], ps[:, :, 0:AIN])
        # one SBUF->SBUF DMA per anchor block moves all 5 planes into the
        # (a_blk, j) layout: dest [8, 5, 384] rows are the 8 samples.
        for i in range(HB2):
            ab = h * HB + hp * HB2 + i
            nc.sync.dma_start(
                out=red5[ab * 8:ab * 8 + 8, :, :],
                in_=tmp5[:, :, i, :],
            )


def _build_program(n_slot_chunks):
    nc = bacc.Bacc(
        "TRN2",
        target_bir_lowering=False,
        debug=False,
        enable_asserts=False,
        num_devices=NCORES,
    )

    def dram_in(name, shape):
        return nc.dram_tensor(name, list(shape), F32, kind="ExternalInput").ap()

    cls_p = dram_in("cls_p", [B, APAD, C])
    ecls_p = dram_in("ecls_p", [B, APAD, C])
    reg_p = dram_in("reg_p", [B, APAD, 4])
    anch16 = dram_in("anch16", [ABLK, 5 * AIN])      # x1,y1,x2,y2,areaA planes
    aux_p = dram_in("aux_p", [128, 6 * AIN])         # acx,acy,r10aw,r10ah,l5aw,l5ah
    slot_sc = dram_in("slot_sc", [128, 5 * n_slot_chunks])
    selbc = dram_in("selbc", [ABLK, 8 * 128])
    selred = dram_in("selred", [128, 5 * 8 * n_slot_chunks])
    selab9 = dram_in("selab9", [128, 128])           # 0.9 * (a_blk match)
    selj = dram_in("selj", [128, 8])
    has_rep = dram_in("has_rep", [128, 1])

    partials = nc.dram_tensor("partials", [8, 3], F32, kind="ExternalOutput").ap()

    with tile.TileContext(nc) as tc, ExitStack() as ctx:
        cpool = ctx.enter_context(tc.tile_pool(name="consts", bufs=1))
        iopool = ctx.enter_context(tc.tile_pool(name="iou", bufs=1))
        wpool = ctx.enter_context(tc.tile_pool(name="work", bufs=1))
        ppool = ctx.enter_context(tc.tile_pool(name="ps", bufs=2, space="PSUM"))
        pspool = ctx.enter_context(tc.tile_pool(name="psR", bufs=2, space="PSUM"))

        # ---- small constants ----
        def cload(name, src, shape):
            t = cpool.tile(shape, F32, name=name)
            nc.sync.dma_start(out=t[:], in_=src[:])
            return t

        def cload_hop(name, src, shape):
            # stationary matmul operands take a DVE hop after the DMA so
            # every matmul's waits collapse onto the single DVE semaphore
            # (walrus rejects PE instructions with >1 sync wait).
            s = cpool.tile(shape, F32, name=name + "_s",
                           tag="hopstage")
            nc.sync.dma_start(out=s[:], in_=src[:])
            t = cpool.tile(shape, F32, name=name)
            nc.vector.tensor_copy(t[:], s[:])
            return t

        anch16_t = cload_hop("anch16_t", anch16, [ABLK, 5 * AIN])
        aux_t = cload("aux_t", aux_p, [128, 6 * AIN])
        slot_t = cload("slot_t", slot_sc, [128, 5 * n_slot_chunks])
        selbc_t = cload_hop("selbc_t", selbc, [ABLK, 8 * 128])
        selred_t = cload_hop("selred_t", selred, [128, 5 * 8 * n_slot_chunks])
        selab_t = cload_hop("selab_t", selab9, [128, 128])
        selj_t = cload_hop("selj_t", selj, [128, 8])
        has_t = cload("has_t", has_rep, [128, 1])

        # ---- stages A-C: anchor broadcast + IoU + slot contraction ----
        # red5 planes: 0=cnt_comb, 1=gw, 2=sx, 3=gh, 4=sy
        red5 = wpool.tile([128, 5, AIN], F32, bufs=1)
        for hp in range(2):
            _emit_iou(nc, iopool, wpool, ppool, pspool, anch16_t, selbc_t,
                      slot_t, selred_t, red5, n_slot_chunks, hp)
        red_t = {n: red5[:, k, :] for k, n in enumerate(
            ["cnt_comb", "gw_r", "sx_r", "gh_r", "sy_r"])}

        # ---- stage D: pos / assigned / npos ----
        pos = wpool.tile([128, AIN], F32, bufs=1)
        nc.vector.tensor_scalar(pos[:], red_t["cnt_comb"][:], 64.0, None,
                                op0=OP.is_ge)
        none_t = wpool.tile([128, AIN], F32, bufs=1)
        nc.vector.tensor_scalar(none_t[:], red_t["cnt_comb"][:], 64.0, 0.0,
                                op0=OP.mod, op1=OP.is_equal)
        # assigned = max(pos, none) * has
        nc.vector.tensor_tensor(none_t[:], pos[:], none_t[:], op=OP.max)
        assigned = wpool.tile([128, AIN], F32, bufs=1)
        nc.vector.tensor_scalar(assigned[:], none_t[:], has_t[:, 0:1], None,
                                op0=OP.mult)
        posacc = wpool.tile([128, 1], F32, bufs=1)
        nc.vector.tensor_reduce(posacc[:], pos[:], axis=AX.X, op=OP.add)

        # ---- stage E: classification loss ----
        clsacc = wpool.tile([128, 1], F32, bufs=1)
        nc.vector.memset(clsacc[:], 0.0)
        NQ = 8
        QW = AIN // NQ  # 48 anchors per chunk

        def _cls_src_ap(base, q, qw):
            dims = [[AIN * C, ABLK], [APAD * C, B], [C, qw], [1, C]]
            return bass.AP(base.tensor, q * qw * C, dims)

        for q in range(NQ):
            sl = slice(q * QW, (q + 1) * QW)
            clsq = wpool.tile([128, QW, C], F32, tag="cA", bufs=2)
            nc.sync.dma_start(out=clsq[:], in_=_cls_src_ap(cls_p, q, QW))
            eclsq = wpool.tile([128, QW, C], F32, tag="cB", bufs=2)
            nc.sync.dma_start(out=eclsq[:], in_=_cls_src_ap(ecls_p, q, QW))

            cc = wpool.tile([128, QW, C], F32, tag="cC", bufs=1)
            nc.vector.tensor_scalar(cc[:], clsq[:], EPS, 1.0 - EPS,
                                    op0=OP.max, op1=OP.min)
            ec = wpool.tile([128, QW, C], F32, tag="cD", bufs=1)
            nc.vector.tensor_scalar(ec[:], eclsq[:], EPS, 1.0 - EPS,
                                    op0=OP.max, op1=OP.min)
            l1 = wpool.tile([128, QW, C], F32, tag="cE", bufs=1)
            nc.scalar.activation(l1[:], cc[:], AF.Ln)
            l0 = wpool.tile([128, QW, C], F32, tag="cF", bufs=1)
            nc.scalar.activation(l0[:], cc[:], AF.Ln, bias=1.0, scale=-1.0)

            # psS = 0.9 * sum_j ecls (selector carries the 0.9).  Reads the
            # clipped tile (DVE-produced, keeps matmul waits at 1); the clip
            # perturbs alpha by <=1e-4 on ~1e-4 of elements (~1e-9 relative).
            eclsq2 = ec.rearrange("p a c -> p (a c)")
            CW = (QW * C) // 2  # 480
            psS = []
            for cch in range(2):
                pst = pspool.tile([128, CW], F32, tag="psS", bufs=2,
                                  name=f"psS{cch}")
                csl = slice(cch * CW, (cch + 1) * CW)
                nc.tensor.matmul(pst[:], lhsT=selab_t[:],
                                 rhs=eclsq2[:, csl], start=True, stop=True)
                psS.append(pst)

            d = clsq  # clsq dead; reuse its slot
            nc.vector.tensor_tensor(d[:], ec[:], cc[:], op=OP.subtract)
            dd = cc  # cc dead after d
            nc.scalar.activation(dd[:], d[:], AF.Square)
            tdif = wpool.tile([128, QW, C], F32, tag="cH", bufs=1)
            nc.vector.tensor_tensor(tdif[:], l1[:], l0[:], op=OP.subtract)
            u = l1  # l1 dead
            nc.vector.tensor_tensor(u[:], ec[:], tdif[:], op=OP.mult)
            bn = tdif  # in place: bn = u + l0
            nc.vector.tensor_tensor(bn[:], u[:], l0[:], op=OP.add)
            w1 = l0  # l0 dead
            nc.vector.tensor_tensor(w1[:], dd[:], bn[:], op=OP.mult)
            # w2 = w1 * (psS + 0.4)
            w2 = ec  # ec dead
            QH = QW // 2
            for cch in range(2):
                asl = slice(cch * QH, (cch + 1) * QH)
                psS3 = psS[cch].rearrange("p (a c) -> p a c", c=C)
                nc.vector.scalar_tensor_tensor(w2[:, asl, :], psS3[:], 0.4,
                                               w1[:, asl, :],
                                               op0=OP.add, op1=OP.mult)
            # w3 = w2 * assigned (+ row-sum accumulate)
            w3 = eclsq  # eclsq dead
            qacc = wpool.tile([128, 1], F32, tag="qacc", bufs=2)
            asg_b = _bc(assigned[:, sl], 2, C)
            nc.vector.scalar_tensor_tensor(w3[:], w2[:], 1.0, asg_b,
                                           op0=OP.mult, op1=OP.mult,
                                           accum_out=qacc[:])
            nc.vector.tensor_tensor(clsacc[:], clsacc[:], qacc[:], op=OP.add)

        # ---- stage F: regression loss (in place in red_t tiles) ----
        regt = wpool.tile([128, AIN, 4], F32, bufs=1)
        reg_src = bass.AP(reg_p.tensor, 0,
                          [[AIN * 4, ABLK], [APAD * 4, B], [4, AIN], [1, 4]])
        nc.sync.dma_start(out=regt[:], in_=reg_src)
        acx = aux_t[:, 0 * AIN:1 * AIN]
        acy = aux_t[:, 1 * AIN:2 * AIN]
        r10aw = aux_t[:, 2 * AIN:3 * AIN]
        r10ah = aux_t[:, 3 * AIN:4 * AIN]
        l5aw = aux_t[:, 4 * AIN:5 * AIN]
        l5ah = aux_t[:, 5 * AIN:6 * AIN]

        tvals = []
        for (gsz, l5) in ((red_t["gw_r"], l5aw), (red_t["gh_r"], l5ah)):
            nc.vector.tensor_scalar(gsz[:], gsz[:], 1.0, None, op0=OP.max)
            nc.scalar.activation(gsz[:], gsz[:], AF.Ln)
            nc.vector.scalar_tensor_tensor(gsz[:], gsz[:], 5.0, l5,
                                           op0=OP.mult, op1=OP.subtract)
            tvals.append(gsz)
        for (s, ac, r10) in ((red_t["sx_r"], acx, r10aw),
                             (red_t["sy_r"], acy, r10ah)):
            nc.vector.scalar_tensor_tensor(s[:], s[:], 0.5, ac,
                                           op0=OP.mult, op1=OP.subtract)
            nc.vector.tensor_tensor(s[:], s[:], r10, op=OP.mult)
            tvals.append(s)
        dw_t, dh_t, dx_t, dy_t = tvals

        for k, tk in enumerate((dx_t, dy_t, dw_t, dh_t)):
            nc.vector.tensor_tensor(tk[:], tk[:], regt[:, :, k], op=OP.subtract)
            nc.scalar.activation(tk[:], tk[:], AF.Abs)
            z = wpool.tile([128, AIN], F32, tag="fz", bufs=2, name=f"z{k}")
            nc.vector.tensor_scalar(z[:], tk[:], -1.0 / 9.0, 0.0,
                                    op0=OP.add, op1=OP.max)
            nc.vector.tensor_scalar(tk[:], tk[:], 1.0 / 9.0, None, op0=OP.min)
            nc.vector.tensor_tensor(tk[:], tk[:], tk[:], op=OP.mult)
            nc.vector.scalar_tensor_tensor(tk[:], tk[:], 4.5, z[:],
                                           op0=OP.mult, op1=OP.add)
        nc.vector.tensor_tensor(dx_t[:], dx_t[:], dy_t[:], op=OP.add)
        nc.vector.tensor_tensor(dw_t[:], dw_t[:], dh_t[:], op=OP.add)
        nc.vector.tensor_tensor(dx_t[:], dx_t[:], dw_t[:], op=OP.add)
        junk = wpool.tile([128, AIN], F32, bufs=1)
        regacc = wpool.tile([128, 1], F32, bufs=1)
        nc.vector.scalar_tensor_tensor(junk[:], dx_t[:], 1.0, pos[:],
                                       op0=OP.mult, op1=OP.mult,
                                       accum_out=regacc[:])

        # ---- stage G: fold to per-sample partials ----
        accs = wpool.tile([128, 3], F32, bufs=1)
        nc.vector.tensor_copy(accs[:, 0:1], posacc[:])
        nc.vector.tensor_copy(accs[:, 1:2], clsacc[:])
        nc.vector.tensor_copy(accs[:, 2:3], regacc[:])
        psF = ppool.tile([8, 3], F32, tag="psA")
        nc.tensor.matmul(psF[:], lhsT=selj_t[:], rhs=accs[:],
                         start=True, stop=True)
        outt = wpool.tile([8, 3], F32, bufs=1)
        nc.scalar.copy(outt[:], psF[:])
        nc.sync.dma_start(out=partials[:], in_=outt[:])

    nc.compile()
    return nc


def _host_prep(inputs, core):
    """Per-core input map (numpy only, O(A) data marshaling)."""
    cls = np.asarray(inputs["classifications"])
    ecls = np.asarray(inputs["ema_classifications"])
    reg = np.asarray(inputs["regressions"])
    anchors = np.asarray(inputs["anchors"])[0]
    a0, a1 = core * ACORE, (core + 1) * ACORE

    def pad_slice(x, fill, k):
        out = np.full((B, APAD, k), fill, dtype=np.float32)
        out[:, :ACORE] = x[:, a0:a1]
        return np.ascontiguousarray(out)

    cls_p = pad_slice(cls, 0.5, C)
    ecls_p = pad_slice(ecls, 0.5, C)
    reg_p = pad_slice(reg, 0.0, 4)

    an = np.zeros((APAD, 4), dtype=np.float32)
    an[:ACORE] = anchors[a0:a1]
    # pad anchors far from every real box: never matched (pos=0),
    # and their cls contribution is 0 because cls==ecls==0.5 there.
    an[ACORE:] = [-4000.0, -4000.0, -3992.0, -3992.0]
    aw = an[:, 2] - an[:, 0]
    ah = an[:, 3] - an[:, 1]
    areaA = aw * ah
    anch16 = np.empty((ABLK, 5 * AIN), dtype=np.float32)
    for pl, arr in enumerate([an[:, 0], an[:, 1], an[:, 2], an[:, 3], areaA]):
        anch16[:, pl * AIN:(pl + 1) * AIN] = arr.reshape(ABLK, AIN)

    acx = (an[:, 0] + 0.5 * aw).astype(np.float32)
    acy = (an[:, 1] + 0.5 * ah).astype(np.float32)
    r10aw = (10.0 / aw).astype(np.float32)
    r10ah = (10.0 / ah).astype(np.float32)
    l5aw = (5.0 * np.log(aw)).astype(np.float32)
    l5ah = (5.0 * np.log(ah)).astype(np.float32)
    aux = np.empty((128, 6 * AIN), dtype=np.float32)
    for pl, arr in enumerate([acx, acy, r10aw, r10ah, l5aw, l5ah]):
        aux[:, pl * AIN:(pl + 1) * AIN] = np.repeat(
            arr.reshape(ABLK, 1, AIN), 8, axis=1).reshape(128, AIN)

    # selbc: block il's coords -> the 64 slot-partitions of each half
    selbc = np.zeros((ABLK, 8 * 128), dtype=np.float32)
    for il in range(HB):
        for h in range(2):
            selbc[h * HB + il, il * 128 + h * 64: il * 128 + (h + 1) * 64] = 1.0

    # selab9: alpha-sum selector with the 0.9 factor folded in
    selab9 = np.zeros((128, 128), dtype=np.float32)
    for ab in range(ABLK):
        selab9[ab * 8:(ab + 1) * 8, ab * 8:(ab + 1) * 8] = 0.9
    selj = np.zeros((128, 8), dtype=np.float32)
    for p in range(128):
        selj[p, p % 8] = 1.0
    return dict(cls_p=cls_p, ecls_p=ecls_p, reg_p=reg_p, anch16=anch16,
                aux_p=aux, selbc=selbc, selab9=selab9, selj=selj)


def _host_slots(inputs):
    """Global (sample, box) slots + selectors (shared by all cores)."""
    ema_classes = np.asarray(inputs["ema_classes"])
    ema_counts = np.asarray(inputs["ema_counts"])
    ema_bboxes = np.asarray(inputs["ema_bboxes"])
    ann = np.asarray(inputs["annotations"])
    M = ema_classes.shape[1]
    valid = np.arange(M)[None, :] < ema_counts[:, None]
    member = (ema_classes[:, :, None] == ann[:, None, :, 4]).any(-1)
    keep = valid & member
    has = keep.any(1)

    slots = [(j, m) for j in range(B) for m in range(M) if keep[j, m]]
    n_chunks = max(1, math.ceil(len(slots) / NS))
    total = n_chunks * NS
    sx1 = np.full(total, -4000.0, np.float32)
    sy1 = np.full(total, -4000.0, np.float32)
    sx2 = np.full(total, -4000.0, np.float32)
    sy2 = np.full(total, -4000.0, np.float32)
    sj = np.full(total, -1, np.int64)
    for idx, (j, m) in enumerate(slots):
        sx1[idx], sy1[idx], sx2[idx], sy2[idx] = ema_bboxes[j, m]
        sj[idx] = j
    areaB = (sx2 - sx1) * (sy2 - sy1)
    areaB[sj < 0] = 0.0

    slot_sc = np.zeros((128, 5 * n_chunks), dtype=np.float32)
    for sc in range(n_chunks):
        seg = slice(sc * NS, (sc + 1) * NS)
        for pl, arr in enumerate([sx1, sy1, sx2, sy2, areaB]):
            col = sc * 5 + pl
            slot_sc[0:64, col] = arr[seg]
            slot_sc[64:128, col] = arr[seg]

    # selred planes per chunk: ones, x2-x1, x1+x2, y2-y1, y1+y2 -> j column
    selred = np.zeros((NS, 5 * 8 * n_chunks), dtype=np.float32)
    for sc in range(n_chunks):
        for s in range(NS):
            g = sc * NS + s
            gj = sj[g]
            if gj < 0:
                continue
            base = sc * 5
            selred[s, (base + 0) * 8 + gj] = 1.0
            selred[s, (base + 1) * 8 + gj] = sx2[g] - sx1[g]
            selred[s, (base + 2) * 8 + gj] = sx1[g] + sx2[g]
            selred[s, (base + 3) * 8 + gj] = sy2[g] - sy1[g]
            selred[s, (base + 4) * 8 + gj] = sy1[g] + sy2[g]
    selred = np.concatenate([selred, selred], axis=0)

    has_rep = np.zeros((128, 1), dtype=np.float32)
    for p in range(128):
        has_rep[p, 0] = 1.0 if has[p % 8] else 0.0
    return n_chunks, slot_sc, selred, has_rep


def _combine(parts):
    """parts: [cores, 8, 3] -> (cls_loss[1], reg_loss[1]) fp32."""
    tot = parts.sum(axis=0).astype(np.float32)
    npos = tot[:, 0]
    clsnum = -tot[:, 1]
    regsum = tot[:, 2]
    denom = np.maximum(npos, 1.0).astype(np.float32)
    cls_loss = (clsnum / denom).astype(np.float32)
    reg_loss = np.where(npos > 0, regsum / (4.0 * denom), 0.0).astype(np.float32)
    return (np.array([cls_loss.mean()], dtype=np.float32),
            np.array([reg_loss.mean()], dtype=np.float32))


def make_in_maps(inputs):
    n_chunks, slot_sc, selred, has_rep = _host_slots(inputs)
    in_maps = []
    for core in range(NCORES):
        m = _host_prep(inputs, core)
        m["slot_sc"] = slot_sc
        m["selred"] = selred
        m["has_rep"] = has_rep
        in_maps.append(m)
    return n_chunks, in_maps


def _run(inputs, **spmd_kwargs):
    n_chunks, in_maps = make_in_maps(inputs)
    if n_chunks not in _prog_cache:
        _prog_cache[n_chunks] = _build_program(n_chunks)
    nc = _prog_cache[n_chunks]
    res = run_bass_kernel_spmd(nc, in_maps, core_ids=list(range(NCORES)),
                               **spmd_kwargs)
    parts = np.stack([r["partials"] for r in res.results])
    return _combine(parts), res


def run_profiled(inputs, **spmd_kwargs):
    """Run with NTFF tracing; returns (outputs, BassKernelResults)."""
    return _run(inputs, trace=True, **spmd_kwargs)


def kernel(**inputs):
    out, _ = _run(inputs)
    return out
